# revision 9
# baseline (speedup 1.0000x reference)
"""Trainium2 Bass kernel for a pre-norm transformer block (attention + MLP).

Shapes: x [4, 1024, 1024], H=16 heads, Dh=64, MLP hidden 4096, f32.

Strategy (8 NeuronCores, no collectives):
  - Token-sharded: core c handles batch row b=c//2, query tokens
    [off, off+512), off=(c%2)*512. Both cores of a pair redundantly
    compute K/V over the full 1024-token row (avoids any cross-core
    communication); everything else is perfectly sharded.
  - Activations flow in transposed layout [feature(partition), token(free)];
    weights are transposed on the host so every matmul streams naturally.
  - Per-token LayerNorm stats via ones-vector matmuls on the PE
    (partition-dim reductions), broadcast back with K=1 matmuls.
  - Softmax denominator folded into the attention A@V matmul via an
    appended ones-column on V (row 64 of the PSUM output is sum(exp)).
  - Matmuls in float32r (TF32-class, full PE rate at N>=256); the
    exp(S)@[V|1] stage runs in bf16.
"""

import sys

try:
    import concourse  # noqa: F401
except ImportError:  # pragma: no cover
    sys.path.insert(0, "/opt/trn_rl_repo")

import numpy as np

import concourse.bass as bass  # noqa: F401
import concourse.tile as tile
from concourse import bacc, bass_utils, mybir

F32 = mybir.dt.float32
F32R = mybir.dt.float32r
BF16 = mybir.dt.bfloat16
AF = mybir.ActivationFunctionType
OP = mybir.AluOpType

P = 128
C = 1024
N = 1024
B = 4
H = 16
DH = 64
DFF = 4096
TOK = 512          # per-core query-token block
NCT = C // P       # 8 c-tiles
NFT = DFF // P     # 32 d'-tiles
EPS = 1e-5
SCALE = DH ** -0.5

_CACHE = {}


def build():
    nc = bacc.Bacc(
        "TRN2",
        target_bir_lowering=False,
        debug=False,
        enable_asserts=False,
        num_devices=8,
    )

    def din(name, shape, dt=F32R):
        return nc.dram_tensor(name, shape, dt, kind="ExternalInput").ap()

    xrow = din("xrow", [C, N])            # x[b].T          (f32r)
    xq = din("xq", [C, TOK])              # query-token slice of x[b].T
    wqkv = din("wqkv", [C, 3 * C])        # qkv_w.T  (Q 0:1024, K 1024:2048, V 2048:3072)
    wproj = din("wproj", [C, C])          # proj_w.T
    wfc1 = din("wfc1", [C, DFF])          # fc1_w.T
    wfc2 = din("wfc2", [DFF, C])          # fc2_w.T
    g1 = din("g1", [P, NCT], F32)
    b1 = din("b1", [P, NCT], F32)
    g2 = din("g2", [P, NCT], F32)
    b2 = din("b2", [P, NCT], F32)
    gh = din("gh", [P, NFT], F32)
    bh = din("bh", [P, NFT], F32)
    pb = din("pb", [P, NCT], F32)
    f1b = din("f1b", [P, NFT], F32)
    f2b = din("f2b", [P, NCT], F32)
    ones1_d = din("ones1", [1, P])        # K=1 broadcast lhsT
    ones128_d = din("ones128", [P, 1])    # partition-sum lhsT

    outT = nc.dram_tensor("outT", [C, TOK], F32, kind="ExternalOutput").ap()

    with tile.TileContext(nc) as tc:
        const = tc.alloc_tile_pool(name="const", bufs=1)
        big = tc.alloc_tile_pool(name="big", bufs=1)
        tmp = tc.alloc_tile_pool(name="tmp", bufs=2)
        misc = tc.alloc_tile_pool(name="misc", bufs=1)
        wpool = tc.alloc_tile_pool(name="w", bufs=8)

        # --- constants ---
        ones1 = const.tile([1, P], F32R)
        nc.sync.dma_start(ones1[:], ones1_d[:])
        ones1h = const.tile([DH + 1, P], F32R)   # ones row AT partition 64
        nc.sync.dma_start(ones1h[DH:DH + 1, :], ones1_d[:])
        ones128 = const.tile([P, 1], F32R)
        nc.sync.dma_start(ones128[:], ones128_d[:])
        eps = const.tile([1, 1], F32)
        nc.vector.memset(eps[:], EPS)
        gb = {}
        for nm, ap_, w in (("g1", g1, NCT), ("b1", b1, NCT), ("g2", g2, NCT),
                           ("b2", b2, NCT), ("gh", gh, NFT), ("bh", bh, NFT),
                           ("pb", pb, NCT), ("f1b", f1b, NFT), ("f2b", f2b, NCT)):
            t = const.tile([P, w], F32, name=nm, tag=nm)
            nc.sync.dma_start(t[:], ap_[:])
            gb[nm] = t

        def ln_stats(stat_ps, bc_ps, src_tiles, n_ct):
            """Per-token LN stats; returns (mu_b, rstd_b) PSUM broadcasts."""
            ps_s = stat_ps.tile([1, TOK], F32, tag="ln_s")
            ps_q = stat_ps.tile([1, TOK], F32, tag="ln_q")
            for ci in range(n_ct):
                s = src_tiles(ci)
                sq = tmp.tile([P, TOK], F32R, tag="ln_sq")
                nc.scalar.activation(sq[:], s, AF.Square)
                nc.tensor.matmul(ps_s[:], ones128[:], s,
                                 start=(ci == 0), stop=(ci == n_ct - 1))
                nc.tensor.matmul(ps_q[:], ones128[:], sq[:],
                                 start=(ci == 0), stop=(ci == n_ct - 1))
            inv = 1.0 / (n_ct * P)
            mu = misc.tile([1, TOK], F32R, tag="ln_mu", bufs=2)
            nc.vector.tensor_scalar_mul(mu[:], ps_s[:], inv)
            ex2 = misc.tile([1, TOK], F32, tag="ln_ex2")
            nc.vector.tensor_scalar_mul(ex2[:], ps_q[:], inv)
            mu2 = misc.tile([1, TOK], F32, tag="ln_mu2")
            nc.vector.tensor_mul(mu2[:], mu[:], mu[:])
            var = misc.tile([1, TOK], F32, tag="ln_var")
            nc.vector.tensor_sub(var[:], ex2[:], mu2[:])
            lnv = misc.tile([1, TOK], F32, tag="ln_lnv")
            nc.scalar.activation(lnv[:], var[:], AF.Ln, bias=eps[:])
            rstd = misc.tile([1, TOK], F32R, tag="ln_rstd", bufs=2)
            nc.scalar.activation(rstd[:], lnv[:], AF.Exp, scale=-0.5)
            mu_b = bc_ps.tile([P, TOK], F32, tag="ln_mub")
            nc.tensor.matmul(mu_b[:], ones1[:], mu[:], start=True, stop=True)
            rstd_b = bc_ps.tile([P, TOK], F32, tag="ln_rstdb")
            nc.tensor.matmul(rstd_b[:], ones1[:], rstd[:], start=True, stop=True)
            return mu_b, rstd_b

        def ln_apply(src, mu_b, rstd_b, g, b, out):
            """out = (src - mu_b) * rstd_b * g + b  (in-place on out after sub)."""
            nc.vector.tensor_sub(out, src, mu_b[:])
            nc.vector.tensor_mul(out, out, rstd_b[:])
            nc.vector.tensor_scalar(out, out, scalar1=g, scalar2=b,
                                    op0=OP.mult, op1=OP.add)

        # --- load x ---
        xr = big.tile([P, NCT, N], F32R, tag="A")      # x[b].T tiled
        nc.sync.dma_start(xr[:], xrow.rearrange("(i p) t -> p i t", p=P))
        xqt = big.tile([P, NCT, TOK], F32R, tag="D")   # query slice (residual)
        nc.sync.dma_start(xqt[:], xq.rearrange("(i p) t -> p i t", p=P))

        # --- LN1 (2 row blocks + query block) ---
        x1n = big.tile([P, NCT, N], F32R, tag="B")
        x1nq = big.tile([P, NCT, TOK], F32R, tag="C")
        ps_stat = tc.alloc_tile_pool(name="ps_stat1", bufs=2, space="PSUM")
        ps_bc = tc.alloc_tile_pool(name="ps_bc1", bufs=2, space="PSUM")
        for blk in range(2):
            sl = slice(blk * TOK, (blk + 1) * TOK)
            mu_b, rstd_b = ln_stats(ps_stat, ps_bc, lambda ci: xr[:, ci, sl], NCT)
            for ci in range(NCT):
                ln_apply(xr[:, ci, sl], mu_b, rstd_b,
                         gb["g1"][:, ci:ci + 1], gb["b1"][:, ci:ci + 1],
                         x1n[:, ci, sl])
        mu_b, rstd_b = ln_stats(ps_stat, ps_bc, lambda ci: xqt[:, ci, :], NCT)
        for ci in range(NCT):
            ln_apply(xqt[:, ci, :], mu_b, rstd_b,
                     gb["g1"][:, ci:ci + 1], gb["b1"][:, ci:ci + 1],
                     x1nq[:, ci, :])
        ps_bc.release()
        ps_stat.release()

        # --- QKV projections (transposed QT/KT, natural V with ones column) ---
        KT = big.tile([P, NCT, N], F32R, tag="A")      # reuses xr slot
        QT = big.tile([P, NCT, TOK], F32R, tag="F")
        V = big.tile([P, NCT, H, DH + 1], BF16, tag="V")
        for r in range(NCT):
            nc.vector.memset(V[:, r, :, DH:DH + 1], 1.0)

        ps_acc = tc.alloc_tile_pool(name="ps_qkv", bufs=4, space="PSUM")
        for g in range(6):  # 512-col groups of wqkv
            wt = []
            for ci in range(NCT):
                w = wpool.tile([P, 512], F32R, tag="w")
                nc.sync.dma_start(w[:], wqkv[ci * P:(ci + 1) * P, g * 512:(g + 1) * 512])
                wt.append(w)
            if g < 2:  # Q -> QT
                for jt in range(4):
                    jj = g * 4 + jt
                    ps = ps_acc.tile([P, TOK], F32, tag="acc")
                    for ci in range(NCT):
                        nc.tensor.matmul(ps[:], wt[ci][:, jt * P:(jt + 1) * P],
                                         x1nq[:, ci, :], start=(ci == 0), stop=(ci == NCT - 1))
                    nc.vector.tensor_copy(QT[:, jj, :], ps[:])
            elif g < 4:  # K -> KT
                for jt in range(4):
                    jj = (g - 2) * 4 + jt
                    for blk in range(2):
                        ps = ps_acc.tile([P, TOK], F32, tag="acc")
                        for ci in range(NCT):
                            nc.tensor.matmul(ps[:], wt[ci][:, jt * P:(jt + 1) * P],
                                             x1n[:, ci, blk * TOK:(blk + 1) * TOK],
                                             start=(ci == 0), stop=(ci == NCT - 1))
                        nc.vector.tensor_copy(KT[:, jj, blk * TOK:(blk + 1) * TOK], ps[:])
            else:  # V natural: lhsT = x1n tile, rhs = w tile
                h0 = 8 * (g - 4)
                for r in range(NCT):
                    ps = ps_acc.tile([P, TOK], F32, tag="acc")
                    for ci in range(NCT):
                        nc.tensor.matmul(ps[:], x1n[:, ci, r * P:(r + 1) * P],
                                         wt[ci][:], start=(ci == 0), stop=(ci == NCT - 1))
                    nc.vector.tensor_copy(
                        V[:, r, h0:h0 + 8, 0:DH],
                        ps[:].rearrange("p (h d) -> p h d", h=8))
        ps_acc.release()

        # --- attention (head pairs share a KT/QT c-tile) ---
        E = big.tile([P, NCT, 2 * TOK], BF16, tag="B")  # reuses x1n slot
        OT = big.tile([P, NCT, TOK], F32R, tag="C")     # reuses x1nq slot
        ps_sa = tc.alloc_tile_pool(name="ps_sa", bufs=2, space="PSUM")
        ps_sb = tc.alloc_tile_pool(name="ps_sb", bufs=2, space="PSUM")
        ps_o = tc.alloc_tile_pool(name="ps_o", bufs=2, space="PSUM")
        ps_l = tc.alloc_tile_pool(name="ps_l", bufs=2, space="PSUM")
        for jj in range(NCT):
            for kt in range(NCT):
                ks = slice(kt * P, (kt + 1) * P)
                psa = ps_sa.tile([P, TOK], F32, tag="Sa")
                nc.tensor.matmul(psa[:], KT[0:64, jj, ks], QT[0:64, jj, :],
                                 start=True, stop=True, tile_position=(0, 0))
                nc.scalar.activation(E[:, kt, 0:TOK], psa[:], AF.Exp, scale=SCALE)
                psb = ps_sb.tile([P, TOK], F32, tag="Sb")
                nc.tensor.matmul(psb[:], KT[64:128, jj, ks], QT[64:128, jj, :],
                                 start=True, stop=True, tile_position=(64, 0))
                nc.scalar.activation(E[:, kt, TOK:2 * TOK], psb[:], AF.Exp, scale=SCALE)
            for half in range(2):
                h = 2 * jj + half
                es = slice(half * TOK, (half + 1) * TOK)
                po = ps_o.tile([P, TOK], F32, tag="O")
                for kt in range(NCT):
                    nc.tensor.matmul(po[0:DH + 1, :], V[:, kt, h, :], E[:, kt, es],
                                     start=(kt == 0), stop=(kt == NCT - 1))
                rec = misc.tile([DH + 1, TOK], F32R, tag="rec", bufs=2)
                with nc.allow_low_precision(reason="softmax denom fed to f32r bcast matmul"):
                    nc.vector.reciprocal(rec[DH:DH + 1, :], po[DH:DH + 1, :])
                pl = ps_l.tile([P, TOK], F32, tag="lbc")
                nc.tensor.matmul(pl[0:DH, :], ones1h[DH:DH + 1, 0:DH], rec[DH:DH + 1, :],
                                 start=True, stop=True)
                pls = misc.tile([DH, TOK], F32, tag="pls", bufs=2)
                nc.vector.tensor_copy(pls[:], pl[0:DH, :])
                if half == 0:
                    nc.vector.tensor_mul(OT[0:DH, jj, :], po[0:DH, :], pls[:])
                else:
                    sh = misc.tile([DH, TOK], F32R, tag="shift", bufs=2)
                    nc.vector.tensor_mul(sh[:], po[0:DH, :], pls[:])
                    nc.gpsimd.dma_start(OT[DH:P, jj, :], sh[:])
        for p_ in (ps_l, ps_o, ps_sb, ps_sa):
            p_.release()

        # --- output projection + residual -> x2 ---
        x2 = big.tile([P, NCT, TOK], F32R, tag="G")
        ps_acc = tc.alloc_tile_pool(name="ps_proj", bufs=4, space="PSUM")
        for ig in range(2):
            wt = []
            for ci in range(NCT):
                w = wpool.tile([P, 512], F32R, tag="w")
                nc.sync.dma_start(w[:], wproj[ci * P:(ci + 1) * P, ig * 512:(ig + 1) * 512])
                wt.append(w)
            for i4 in range(4):
                i = ig * 4 + i4
                ps = ps_acc.tile([P, TOK], F32, tag="acc")
                for ci in range(NCT):
                    nc.tensor.matmul(ps[:], wt[ci][:, i4 * P:(i4 + 1) * P],
                                     OT[:, ci, :], start=(ci == 0), stop=(ci == NCT - 1))
                nc.vector.scalar_tensor_tensor(
                    x2[:, i, :], ps[:], gb["pb"][:, i:i + 1], xqt[:, i, :],
                    op0=OP.add, op1=OP.add)
        ps_acc.release()

        # --- LN2 ---
        x2n = big.tile([P, NCT, TOK], F32R, tag="F")   # reuses QT slot
        ps_stat = tc.alloc_tile_pool(name="ps_stat2", bufs=1, space="PSUM")
        ps_bc = tc.alloc_tile_pool(name="ps_bc2", bufs=1, space="PSUM")
        mu_b, rstd_b = ln_stats(ps_stat, ps_bc, lambda ci: x2[:, ci, :], NCT)
        for ci in range(NCT):
            ln_apply(x2[:, ci, :], mu_b, rstd_b,
                     gb["g2"][:, ci:ci + 1], gb["b2"][:, ci:ci + 1],
                     x2n[:, ci, :])
        ps_bc.release()
        ps_stat.release()

        # --- fc1 + gelu -> U ---
        U0 = big.tile([P, NFT // 2, TOK], F32R, tag="A")  # reuses KT slot
        U1 = big.tile([P, NFT // 2, TOK], F32R, tag="B")  # reuses E slot

        def u_tile(i):
            return (U0 if i < NFT // 2 else U1)[:, i % (NFT // 2), :]

        ps_acc = tc.alloc_tile_pool(name="ps_fc1", bufs=4, space="PSUM")
        for ig in range(8):
            wt = []
            for ci in range(NCT):
                w = wpool.tile([P, 512], F32R, tag="w")
                nc.sync.dma_start(w[:], wfc1[ci * P:(ci + 1) * P, ig * 512:(ig + 1) * 512])
                wt.append(w)
            for i4 in range(4):
                i = ig * 4 + i4
                ps = ps_acc.tile([P, TOK], F32, tag="acc")
                for ci in range(NCT):
                    nc.tensor.matmul(ps[:], wt[ci][:, i4 * P:(i4 + 1) * P],
                                     x2n[:, ci, :], start=(ci == 0), stop=(ci == NCT - 1))
                nc.scalar.activation(u_tile(i), ps[:], AF.Gelu,
                                     bias=gb["f1b"][:, i:i + 1])
        ps_acc.release()

        # --- LN on hidden ---
        ps_stat = tc.alloc_tile_pool(name="ps_stath", bufs=1, space="PSUM")
        ps_bc = tc.alloc_tile_pool(name="ps_bch", bufs=1, space="PSUM")
        mu_b, rstd_b = ln_stats(ps_stat, ps_bc, u_tile, NFT)
        mu_s = misc.tile([P, TOK], F32, tag="mu_s")
        nc.vector.tensor_copy(mu_s[:], mu_b[:])
        rstd_s = misc.tile([P, TOK], F32, tag="rstd_s")
        nc.vector.tensor_copy(rstd_s[:], rstd_b[:])
        ps_bc.release()
        ps_stat.release()

        # --- fc2 (streamed over d' with 8 resident accumulators) + residual ---
        ps_fc2 = tc.alloc_tile_pool(name="ps_fc2", bufs=1, space="PSUM")
        fps = [ps_fc2.tile([P, TOK], F32, tag=f"fc2_{j}", name=f"fc2_{j}")
               for j in range(NCT)]
        for i in range(NFT):
            un = tmp.tile([P, TOK], F32R, tag="un")
            ln_apply(u_tile(i), mu_s, rstd_s,
                     gb["gh"][:, i:i + 1], gb["bh"][:, i:i + 1], un[:])
            wa = wpool.tile([P, 512], F32R, tag="w")
            nc.sync.dma_start(wa[:], wfc2[i * P:(i + 1) * P, 0:512])
            wb = wpool.tile([P, 512], F32R, tag="w")
            nc.sync.dma_start(wb[:], wfc2[i * P:(i + 1) * P, 512:1024])
            for j in range(NCT):
                w = wa if j < 4 else wb
                nc.tensor.matmul(fps[j][:], w[:, (j % 4) * P:(j % 4 + 1) * P], un[:],
                                 start=(i == 0), stop=(i == NFT - 1))
        for j in range(NCT):
            ot = tmp.tile([P, TOK], F32, tag="out")
            nc.vector.scalar_tensor_tensor(
                ot[:], fps[j][:], gb["f2b"][:, j:j + 1], x2[:, j, :],
                op0=OP.add, op1=OP.add)
            nc.sync.dma_start(outT[j * P:(j + 1) * P, :], ot[:])
        ps_fc2.release()

        for p_ in (wpool, misc, tmp, big, const):
            p_.release()

    nc.compile()
    return nc


def _prep_inputs(inputs):
    """Host-side transposes/slices -> per-core in_maps."""
    f = lambda a: np.asarray(a, dtype=np.float32)
    x = f(inputs["x"])
    xT = np.ascontiguousarray(x.transpose(0, 2, 1))          # [B, C, N]
    common = {
        "wqkv": np.ascontiguousarray(f(inputs["qkv_w"]).T),
        "wproj": np.ascontiguousarray(f(inputs["proj_w"]).T),
        "wfc1": np.ascontiguousarray(f(inputs["fc1_w"]).T),
        "wfc2": np.ascontiguousarray(f(inputs["fc2_w"]).T),
        "g1": np.ascontiguousarray(f(inputs["ln1_g"]).reshape(NCT, P).T),
        "b1": np.ascontiguousarray(f(inputs["ln1_b"]).reshape(NCT, P).T),
        "g2": np.ascontiguousarray(f(inputs["ln2_g"]).reshape(NCT, P).T),
        "b2": np.ascontiguousarray(f(inputs["ln2_b"]).reshape(NCT, P).T),
        "gh": np.ascontiguousarray(f(inputs["lnh_g"]).reshape(NFT, P).T),
        "bh": np.ascontiguousarray(f(inputs["lnh_b"]).reshape(NFT, P).T),
        "pb": np.ascontiguousarray(f(inputs["proj_b"]).reshape(NCT, P).T),
        "f1b": np.ascontiguousarray(f(inputs["fc1_b"]).reshape(NFT, P).T),
        "f2b": np.ascontiguousarray(f(inputs["fc2_b"]).reshape(NCT, P).T),
        "ones1": np.ones((1, P), np.float32),
        "ones128": np.ones((P, 1), np.float32),
    }
    in_maps = []
    for c in range(8):
        b, off = c // 2, (c % 2) * TOK
        m = dict(common)
        m["xrow"] = np.ascontiguousarray(xT[b])
        m["xq"] = np.ascontiguousarray(xT[b][:, off:off + TOK])
        in_maps.append(m)
    return in_maps


def _assemble(results):
    out = np.empty((B, N, C), np.float32)
    for c in range(8):
        b, off = c // 2, (c % 2) * TOK
        out[b, off:off + TOK, :] = results[c]["outT"].T
    return out


def kernel(**inputs) -> np.ndarray:
    nc = _CACHE.get("nc")
    if nc is None:
        nc = build()
        _CACHE["nc"] = nc
    in_maps = _prep_inputs(inputs)
    res = bass_utils.run_bass_kernel_spmd(nc, in_maps, core_ids=list(range(8)))
    return _assemble(res.results)


# revision 13
# speedup vs baseline: 1.0813x; 1.0813x over previous
"""Trainium2 Bass kernel for a pre-norm transformer block (attention + MLP).

Shapes: x [4, 1024, 1024], H=16 heads, Dh=64, MLP hidden 4096, f32.

Strategy (8 NeuronCores, no collectives):
  - Token-sharded: core c handles batch row b=c//2, query tokens
    [off, off+512), off=(c%2)*512. Both cores of a pair redundantly
    compute K/V over the full 1024-token row (no cross-core comms);
    everything else is perfectly sharded.
  - Activations flow in transposed layout [feature(partition), token(free)];
    weights are transposed on the host so every matmul streams naturally.
  - LayerNorm gains are folded into the following weight matrix on the host
    (biases asserted zero); LN1 is fused algebraically into the QKV
    evictions:  ln(x) @ W'^T = rstd*(x @ W'^T) + (-mu*rstd)*rowsum(W').
  - Per-token LN stats via ones-vector matmuls on the PE (partition-dim
    reductions), broadcast back with K=1 matmuls.
  - Softmax denominator folded into the attention A@V matmul via an
    appended ones-column on V (row 64 of the PSUM output is sum(exp)).
  - Matmuls in float32r (TF32-class, full PE rate); the attention path
    (S^T, exp(S)@[V|1], proj) runs in bf16.
"""

import sys

try:
    import concourse  # noqa: F401
except ImportError:  # pragma: no cover
    sys.path.insert(0, "/opt/trn_rl_repo")

import ml_dtypes
import numpy as np

import concourse.bass as bass  # noqa: F401
import concourse.tile as tile
from concourse import bacc, bass_utils, mybir

F32 = mybir.dt.float32
F32R = mybir.dt.float32r
BF16 = mybir.dt.bfloat16
AF = mybir.ActivationFunctionType
OP = mybir.AluOpType

P = 128
C = 1024
N = 1024
B = 4
H = 16
DH = 64
DFF = 4096
TOK = 512          # per-core query-token block
NCT = C // P       # 8 c-tiles
NFT = DFF // P     # 32 d'-tiles
EPS = 1e-5
SCALE = DH ** -0.5

_CACHE = {}


def build():
    nc = bacc.Bacc(
        "TRN2",
        target_bir_lowering=False,
        debug=False,
        enable_asserts=False,
        num_devices=8,
    )

    def din(name, shape, dt=F32R):
        return nc.dram_tensor(name, shape, dt, kind="ExternalInput").ap()

    xrow = din("xrow", [C, N])            # x[b].T          (f32r)
    xq = din("xq", [C, TOK])              # query-token slice of x[b].T
    wqkv = din("wqkv", [C, 3 * C])        # (qkv_w * ln1_g).T
    wproj = din("wproj", [C, C], BF16)    # proj_w.T (bf16)
    wfc1 = din("wfc1", [C, DFF], BF16)    # (fc1_w * ln2_g).T (bf16)
    wfc2 = din("wfc2", [DFF, C], BF16)    # (fc2_w * lnh_g).T (bf16)
    wqs = din("wqs", [P, 3 * NCT], F32)   # per-col rowsums of folded qkv_w
    wvs = din("wvs", [1, C], F32R)        # rowsums for V cols (row layout)
    pb = din("pb", [P, NCT], F32)
    f1b = din("f1b", [P, NFT], F32)
    f2b = din("f2b", [P, NCT], F32)
    ones1_d = din("ones1", [1, P])        # K=1 broadcast lhsT
    ones128_d = din("ones128", [P, 1])    # partition-sum lhsT

    outT = nc.dram_tensor("outT", [C, TOK], F32, kind="ExternalOutput").ap()

    with tile.TileContext(nc) as tc:
        const = tc.alloc_tile_pool(name="const", bufs=1)
        big = tc.alloc_tile_pool(name="big", bufs=1)
        tmp = tc.alloc_tile_pool(name="tmp", bufs=2)
        misc = tc.alloc_tile_pool(name="misc", bufs=1)
        wpool = tc.alloc_tile_pool(name="w", bufs=9)

        # --- constants ---
        ones1 = const.tile([1, P], F32R)
        nc.sync.dma_start(ones1[:], ones1_d[:])
        ones1h = const.tile([DH + 1, P], F32R)   # ones row AT partition 64
        nc.sync.dma_start(ones1h[DH:DH + 1, :], ones1_d[:])
        ones128 = const.tile([P, 1], F32R)
        nc.sync.dma_start(ones128[:], ones128_d[:])
        ones128b = const.tile([P, 1], BF16)
        nc.vector.memset(ones128b[:], 1.0)
        eps = const.tile([1, 1], F32)
        nc.vector.memset(eps[:], EPS)
        gb = {}
        for nm, ap_, w in (("wqs", wqs, 3 * NCT), ("pb", pb, NCT),
                           ("f1b", f1b, NFT), ("f2b", f2b, NCT)):
            t = const.tile([P, w], F32, name=nm, tag=nm)
            nc.sync.dma_start(t[:], ap_[:])
            gb[nm] = t
        wvs_s = const.tile([1, C], F32R)
        nc.sync.dma_start(wvs_s[:], wvs[:])

        def ln_stats(stat_ps, src_tiles, n_ct, ones_lhs):
            """Returns (mu, rstd) [1, TOK] SBUF rows (f32r)."""
            ps_s = stat_ps.tile([1, TOK], F32, tag="ln_s")
            ps_q = stat_ps.tile([1, TOK], F32, tag="ln_q")
            for ci in range(n_ct):
                s = src_tiles(ci)
                sq = tmp.tile([P, TOK], F32R, tag="ln_sq")
                nc.scalar.activation(sq[:], s, AF.Square)
                nc.tensor.matmul(ps_s[:], ones_lhs[:], s,
                                 start=(ci == 0), stop=(ci == n_ct - 1))
                nc.tensor.matmul(ps_q[:], ones128[:], sq[:],
                                 start=(ci == 0), stop=(ci == n_ct - 1))
            inv = 1.0 / (n_ct * P)
            mu = misc.tile([1, TOK], F32R, tag="ln_mu", bufs=2)
            nc.vector.tensor_scalar_mul(mu[:], ps_s[:], inv)
            ex2 = misc.tile([1, TOK], F32, tag="ln_ex2", bufs=2)
            nc.vector.tensor_scalar_mul(ex2[:], ps_q[:], inv)
            mu2 = misc.tile([1, TOK], F32, tag="ln_mu2", bufs=2)
            nc.vector.tensor_mul(mu2[:], mu[:], mu[:])
            nc.vector.tensor_sub(ex2[:], ex2[:], mu2[:])      # var, in place
            nc.scalar.activation(ex2[:], ex2[:], AF.Ln, bias=eps[:])
            rstd = misc.tile([1, TOK], F32R, tag="ln_rstd", bufs=2)
            nc.scalar.activation(rstd[:], ex2[:], AF.Exp, scale=-0.5)
            return mu, rstd

        # --- load x ---
        xr = big.tile([P, NCT, N], F32R, tag="A")      # x[b].T tiled
        nc.sync.dma_start(xr[:], xrow.rearrange("(i p) t -> p i t", p=P))
        xqt = big.tile([P, NCT, TOK], F32R, tag="D")   # query slice (residual)
        nc.sync.dma_start(xqt[:], xq.rearrange("(i p) t -> p i t", p=P))

        # --- LN1 stats for row blocks and query block; broadcast rstd/-mu*rstd ---
        ps_stat = tc.alloc_tile_pool(name="ps_stat1", bufs=2, space="PSUM")
        ps_bc = tc.alloc_tile_pool(name="ps_bc1", bufs=1, space="PSUM")
        ps_tp = tc.alloc_tile_pool(name="ps_tp", bufs=2, space="PSUM")
        rstd_sb = []   # [128, TOK] f32 per block (0,1 = row blocks, 2 = q)
        nmr_sb = []
        rstdT = misc.tile([P, NCT], F32, tag="rstdT")   # column form per tok-tile
        nmrT = misc.tile([P, NCT], F32, tag="nmrT")
        for blk in range(3):
            if blk < 2:
                sl = slice(blk * TOK, (blk + 1) * TOK)
                mu, rstd = ln_stats(ps_stat, lambda ci: xr[:, ci, sl], NCT, ones128)
            else:
                mu, rstd = ln_stats(ps_stat, lambda ci: xqt[:, ci, :], NCT, ones128)
            nmr = misc.tile([1, TOK], F32R, tag="ln_nmr", bufs=2)
            nc.vector.tensor_mul(nmr[:], mu[:], rstd[:])
            nc.vector.tensor_scalar_mul(nmr[:], nmr[:], -1.0)
            bc_r = ps_bc.tile([P, TOK], F32, tag="bc_r")
            nc.tensor.matmul(bc_r[:], ones1[:], rstd[:], start=True, stop=True)
            bc_n = ps_bc.tile([P, TOK], F32, tag="bc_n")
            nc.tensor.matmul(bc_n[:], ones1[:], nmr[:], start=True, stop=True)
            r_sb = misc.tile([P, TOK], BF16, tag="lnsb", bufs=6, name=f"rsb{blk}")
            nc.vector.tensor_copy(r_sb[:], bc_r[:])
            n_sb = misc.tile([P, TOK], BF16, tag="lnsb", bufs=6, name=f"nsb{blk}")
            nc.vector.tensor_copy(n_sb[:], bc_n[:])
            rstd_sb.append(r_sb)
            nmr_sb.append(n_sb)
            if blk < 2:
                # transpose rstd/nmr rows into per-token-tile columns (for V)
                for sub in range(4):
                    r = blk * 4 + sub
                    cs = slice(sub * P, (sub + 1) * P)
                    pt = ps_tp.tile([P, 1], F32, tag="tp", name=f"tp{r}")
                    nc.tensor.matmul(pt[:], rstd[0:1, cs].bitcast(F32),
                                     ones1[0:1, 0:1].bitcast(F32),
                                     start=True, stop=True)
                    nc.vector.tensor_copy(rstdT[:, r:r + 1], pt[:])
                    pt2 = ps_tp.tile([P, 1], F32, tag="tp", name=f"tp2_{r}")
                    nc.tensor.matmul(pt2[:], nmr[0:1, cs].bitcast(F32),
                                     ones1[0:1, 0:1].bitcast(F32),
                                     start=True, stop=True)
                    nc.vector.tensor_copy(nmrT[:, r:r + 1], pt2[:])
        # broadcast V-column rowsums to all partitions (once)
        wvs_b = misc.tile([P, C], BF16, tag="wvs_b")
        for g in range(2):
            bc = ps_bc.tile([P, TOK], F32, tag="bc_r", name=f"bcv{g}")
            nc.tensor.matmul(bc[:], ones1[:], wvs_s[0:1, g * TOK:(g + 1) * TOK],
                             start=True, stop=True)
            nc.vector.tensor_copy(wvs_b[:, g * TOK:(g + 1) * TOK], bc[:])
        ps_tp.release()
        ps_bc.release()
        ps_stat.release()

        # --- QKV with fused LN1 (transposed QT/KT bf16, natural V bf16 + ones) ---
        KT = big.tile([P, NCT, N], BF16, tag="B")
        QT = big.tile([P, NCT, TOK], BF16, tag="F")
        V = big.tile([P, NCT, H, DH + 1], BF16, tag="V")
        for r in range(NCT):
            nc.vector.memset(V[:, r, :, DH:DH + 1], 1.0)

        ps_acc = tc.alloc_tile_pool(name="ps_qkv", bufs=4, space="PSUM")
        for g in range(6):  # 512-col groups of wqkv
            wt = []
            for ci in range(NCT):
                w = wpool.tile([P, 512], F32R, tag="w")
                nc.sync.dma_start(w[:], wqkv[ci * P:(ci + 1) * P, g * 512:(g + 1) * 512])
                wt.append(w)
            if g < 2:  # Q -> QT (query block, fused LN)
                for jt in range(4):
                    jj = g * 4 + jt
                    ps = ps_acc.tile([P, TOK], F32, tag="acc")
                    for ci in range(NCT):
                        nc.tensor.matmul(ps[:], wt[ci][:, jt * P:(jt + 1) * P],
                                         xqt[:, ci, :], start=(ci == 0), stop=(ci == NCT - 1))
                    t = tmp.tile([P, TOK], BF16, tag="ev", bufs=3)
                    nc.vector.tensor_mul(t[:], ps[:], rstd_sb[2][:])
                    nc.vector.scalar_tensor_tensor(
                        QT[:, jj, :], nmr_sb[2][:], gb["wqs"][:, jj:jj + 1], t[:],
                        op0=OP.mult, op1=OP.add)
            elif g < 4:  # K -> KT
                for jt in range(4):
                    jj = (g - 2) * 4 + jt
                    for blk in range(2):
                        ps = ps_acc.tile([P, TOK], F32, tag="acc")
                        for ci in range(NCT):
                            nc.tensor.matmul(ps[:], wt[ci][:, jt * P:(jt + 1) * P],
                                             xr[:, ci, blk * TOK:(blk + 1) * TOK],
                                             start=(ci == 0), stop=(ci == NCT - 1))
                        t = tmp.tile([P, TOK], BF16, tag="ev", bufs=3)
                        nc.vector.tensor_mul(t[:], ps[:], rstd_sb[blk][:])
                        nc.vector.scalar_tensor_tensor(
                            KT[:, jj, blk * TOK:(blk + 1) * TOK],
                            nmr_sb[blk][:], gb["wqs"][:, NCT + jj:NCT + jj + 1], t[:],
                            op0=OP.mult, op1=OP.add)
            else:  # V natural: lhsT = raw x tile, rhs = w tile; per-token scalars
                h0 = 8 * (g - 4)
                dsl = slice((g - 4) * TOK, (g - 4 + 1) * TOK)
                for r in range(NCT):
                    ps = ps_acc.tile([P, TOK], F32, tag="acc")
                    for ci in range(NCT):
                        nc.tensor.matmul(ps[:], xr[:, ci, r * P:(r + 1) * P],
                                         wt[ci][:], start=(ci == 0), stop=(ci == NCT - 1))
                    t = tmp.tile([P, TOK], BF16, tag="ev", bufs=3)
                    nc.vector.tensor_scalar_mul(t[:], wvs_b[:, dsl], nmrT[:, r:r + 1])
                    nc.vector.scalar_tensor_tensor(
                        V[:, r, h0:h0 + 8, 0:DH],
                        ps[:].rearrange("p (h d) -> p h d", h=8),
                        rstdT[:, r:r + 1],
                        t[:].rearrange("p (h d) -> p h d", h=8),
                        op0=OP.mult, op1=OP.add)
        ps_acc.release()

        # --- attention (head pairs share a KT/QT c-tile) ---
        E = big.tile([P, NCT, 2 * TOK], BF16, tag="E")
        OT = big.tile([P, NCT, TOK], BF16, tag="C")
        ps_sa = tc.alloc_tile_pool(name="ps_sa", bufs=2, space="PSUM")
        ps_sb_ = tc.alloc_tile_pool(name="ps_sb", bufs=2, space="PSUM")
        ps_o = tc.alloc_tile_pool(name="ps_o", bufs=2, space="PSUM")
        ps_l = tc.alloc_tile_pool(name="ps_l", bufs=2, space="PSUM")
        for jj in range(NCT):
            for kt in range(NCT):
                ks = slice(kt * P, (kt + 1) * P)
                psa = ps_sa.tile([P, TOK], F32, tag="Sa")
                nc.tensor.matmul(psa[:], KT[0:64, jj, ks], QT[0:64, jj, :],
                                 start=True, stop=True, tile_position=(0, 0))
                nc.scalar.activation(E[:, kt, 0:TOK], psa[:], AF.Exp, scale=SCALE)
                psb = ps_sb_.tile([P, TOK], F32, tag="Sb")
                nc.tensor.matmul(psb[:], KT[64:128, jj, ks], QT[64:128, jj, :],
                                 start=True, stop=True, tile_position=(64, 0))
                nc.scalar.activation(E[:, kt, TOK:2 * TOK], psb[:], AF.Exp, scale=SCALE)
            for half in range(2):
                h = 2 * jj + half
                es = slice(half * TOK, (half + 1) * TOK)
                po = ps_o.tile([P, TOK], F32, tag="O")
                for kt in range(NCT):
                    nc.tensor.matmul(po[0:DH + 1, :], V[:, kt, h, :], E[:, kt, es],
                                     start=(kt == 0), stop=(kt == NCT - 1))
                rec = misc.tile([DH + 1, TOK], F32R, tag="rec", bufs=2)
                with nc.allow_low_precision(reason="softmax denom to f32r bcast"):
                    nc.vector.reciprocal(rec[DH:DH + 1, :], po[DH:DH + 1, :])
                pl = ps_l.tile([P, TOK], F32, tag="lbc")
                nc.tensor.matmul(pl[0:DH, :], ones1h[DH:DH + 1, 0:DH],
                                 rec[DH:DH + 1, :], start=True, stop=True)
                pls = misc.tile([DH, TOK], BF16, tag="pls", bufs=2)
                nc.vector.tensor_copy(pls[:], pl[0:DH, :])
                if half == 0:
                    nc.vector.tensor_mul(OT[0:DH, jj, :], po[0:DH, :], pls[:])
                else:
                    sh = misc.tile([DH, TOK], BF16, tag="shift", bufs=2)
                    nc.vector.tensor_mul(sh[:], po[0:DH, :], pls[:])
                    nc.gpsimd.dma_start(OT[DH:P, jj, :], sh[:])
        for p_ in (ps_l, ps_o, ps_sb_, ps_sa):
            p_.release()

        # --- output projection (bf16) + residual -> x2 ---
        x2 = big.tile([P, NCT, TOK], F32R, tag="B")   # reuses KT slot
        ps_acc = tc.alloc_tile_pool(name="ps_proj", bufs=4, space="PSUM")
        for ig in range(2):
            wt = []
            for ci in range(NCT):
                w = wpool.tile([P, 512], BF16, tag="wb")
                nc.sync.dma_start(w[:], wproj[ci * P:(ci + 1) * P, ig * 512:(ig + 1) * 512])
                wt.append(w)
            for i4 in range(4):
                i = ig * 4 + i4
                ps = ps_acc.tile([P, TOK], F32, tag="acc")
                for ci in range(NCT):
                    nc.tensor.matmul(ps[:], wt[ci][:, i4 * P:(i4 + 1) * P],
                                     OT[:, ci, :], start=(ci == 0), stop=(ci == NCT - 1))
                nc.vector.scalar_tensor_tensor(
                    x2[:, i, :], ps[:], gb["pb"][:, i:i + 1], xqt[:, i, :],
                    op0=OP.add, op1=OP.add)
        ps_acc.release()

        # --- LN2 (bare; ln2_g folded into wfc1) ---
        x2n = big.tile([P, NCT, TOK], BF16, tag="F")   # reuses QT slot
        ps_stat = tc.alloc_tile_pool(name="ps_stat2", bufs=1, space="PSUM")
        ps_bc = tc.alloc_tile_pool(name="ps_bc2", bufs=1, space="PSUM")
        mu, rstd = ln_stats(ps_stat, lambda ci: x2[:, ci, :], NCT, ones128)
        mu_b = ps_bc.tile([P, TOK], F32, tag="mu_b")
        nc.tensor.matmul(mu_b[:], ones1[:], mu[:], start=True, stop=True)
        rstd_b = ps_bc.tile([P, TOK], F32, tag="rstd_b")
        nc.tensor.matmul(rstd_b[:], ones1[:], rstd[:], start=True, stop=True)
        rb_s = misc.tile([P, TOK], BF16, tag="lnsb", bufs=6, name="rb_s2")
        nc.vector.tensor_copy(rb_s[:], rstd_b[:])
        for ci in range(NCT):
            nc.vector.tensor_sub(x2n[:, ci, :], x2[:, ci, :], mu_b[:])
            nc.vector.tensor_mul(x2n[:, ci, :], x2n[:, ci, :], rb_s[:])
        ps_bc.release()
        ps_stat.release()

        # --- fc1 + gelu -> U (bf16, split over the V and E slots) ---
        U0 = big.tile([P, NFT // 2, TOK], BF16, tag="V")   # reuses V slot
        U1 = big.tile([P, NFT // 2, TOK], BF16, tag="E")   # reuses E slot

        def u_tile(i):
            return (U0 if i < NFT // 2 else U1)[:, i % (NFT // 2), :]

        ps_acc = tc.alloc_tile_pool(name="ps_fc1", bufs=4, space="PSUM")
        for ig in range(8):
            wt = []
            for ci in range(NCT):
                w = wpool.tile([P, 512], BF16, tag="wb")
                nc.sync.dma_start(w[:], wfc1[ci * P:(ci + 1) * P, ig * 512:(ig + 1) * 512])
                wt.append(w)
            for i4 in range(4):
                i = ig * 4 + i4
                ps = ps_acc.tile([P, TOK], F32, tag="acc")
                for ci in range(NCT):
                    nc.tensor.matmul(ps[:], wt[ci][:, i4 * P:(i4 + 1) * P],
                                     x2n[:, ci, :], start=(ci == 0), stop=(ci == NCT - 1))
                nc.scalar.activation(u_tile(i), ps[:], AF.Gelu,
                                     bias=gb["f1b"][:, i:i + 1])
        ps_acc.release()

        # --- LNh stats (bare; lnh_g folded into wfc2) ---
        ps_stat = tc.alloc_tile_pool(name="ps_stath", bufs=1, space="PSUM")
        ps_bc = tc.alloc_tile_pool(name="ps_bch", bufs=1, space="PSUM")
        mu, rstd = ln_stats(ps_stat, u_tile, NFT, ones128b)
        mu_b = ps_bc.tile([P, TOK], F32, tag="mu_bh")
        nc.tensor.matmul(mu_b[:], ones1[:], mu[:], start=True, stop=True)
        rstd_b = ps_bc.tile([P, TOK], F32, tag="rstd_bh")
        nc.tensor.matmul(rstd_b[:], ones1[:], rstd[:], start=True, stop=True)
        mu_s = misc.tile([P, TOK], BF16, tag="lnsb", bufs=6, name="mu_sh")
        nc.vector.tensor_copy(mu_s[:], mu_b[:])
        rstd_s = misc.tile([P, TOK], BF16, tag="lnsb", bufs=6, name="rstd_sh")
        nc.vector.tensor_copy(rstd_s[:], rstd_b[:])
        ps_bc.release()
        ps_stat.release()

        # --- fc2 (streamed over d' with 8 resident accumulators) + residual ---
        ps_fc2 = tc.alloc_tile_pool(name="ps_fc2", bufs=1, space="PSUM")
        fps = [ps_fc2.tile([P, TOK], F32, tag=f"fc2_{j}", name=f"fc2_{j}")
               for j in range(NCT)]
        for i in range(NFT):
            un = tmp.tile([P, TOK], BF16, tag="un")
            nc.vector.tensor_sub(un[:], u_tile(i), mu_s[:])
            nc.vector.tensor_mul(un[:], un[:], rstd_s[:])
            wa = wpool.tile([P, 512], BF16, tag="wb")
            nc.sync.dma_start(wa[:], wfc2[i * P:(i + 1) * P, 0:512])
            wb = wpool.tile([P, 512], BF16, tag="wb")
            nc.sync.dma_start(wb[:], wfc2[i * P:(i + 1) * P, 512:1024])
            for j in range(NCT):
                w = wa if j < 4 else wb
                nc.tensor.matmul(fps[j][:], w[:, (j % 4) * P:(j % 4 + 1) * P], un[:],
                                 start=(i == 0), stop=(i == NFT - 1))
        for j in range(NCT):
            ot = tmp.tile([P, TOK], F32, tag="out")
            nc.vector.scalar_tensor_tensor(
                ot[:], fps[j][:], gb["f2b"][:, j:j + 1], x2[:, j, :],
                op0=OP.add, op1=OP.add)
            nc.sync.dma_start(outT[j * P:(j + 1) * P, :], ot[:])
        ps_fc2.release()

        for p_ in (wpool, misc, tmp, big, const):
            p_.release()

    nc.compile()
    return nc


def _prep_inputs(inputs):
    """Host-side transposes/folds/slices -> per-core in_maps."""
    f = lambda a: np.asarray(a, dtype=np.float32)
    x = f(inputs["x"])
    xT = np.ascontiguousarray(x.transpose(0, 2, 1))          # [B, C, N]

    g1, b1 = f(inputs["ln1_g"]), f(inputs["ln1_b"])
    g2, b2 = f(inputs["ln2_g"]), f(inputs["ln2_b"])
    ghv, bhv = f(inputs["lnh_g"]), f(inputs["lnh_b"])
    for nm, bb in (("ln1_b", b1), ("ln2_b", b2), ("lnh_b", bhv)):
        if np.abs(bb).max() != 0.0:
            raise NotImplementedError(f"{nm} != 0 not supported by this kernel")

    qkv_f = f(inputs["qkv_w"]) * g1[None, :]      # fold ln1_g
    fc1_f = f(inputs["fc1_w"]) * g2[None, :]      # fold ln2_g
    fc2_f = f(inputs["fc2_w"]) * ghv[None, :]     # fold lnh_g
    qs = qkv_f.sum(axis=1)                        # [3072] rowsums

    common = {
        "wqkv": np.ascontiguousarray(qkv_f.T),
        "wproj": np.ascontiguousarray(f(inputs["proj_w"]).T.astype(ml_dtypes.bfloat16)),
        "wfc1": np.ascontiguousarray(fc1_f.T.astype(ml_dtypes.bfloat16)),
        "wfc2": np.ascontiguousarray(fc2_f.T.astype(ml_dtypes.bfloat16)),
        "wqs": np.ascontiguousarray(qs.reshape(3 * NCT, P).T),
        "wvs": np.ascontiguousarray(qs[2 * C:].reshape(1, C)),
        "pb": np.ascontiguousarray(f(inputs["proj_b"]).reshape(NCT, P).T),
        "f1b": np.ascontiguousarray(f(inputs["fc1_b"]).reshape(NFT, P).T),
        "f2b": np.ascontiguousarray(f(inputs["fc2_b"]).reshape(NCT, P).T),
        "ones1": np.ones((1, P), np.float32),
        "ones128": np.ones((P, 1), np.float32),
    }
    in_maps = []
    for c in range(8):
        b, off = c // 2, (c % 2) * TOK
        m = dict(common)
        m["xrow"] = np.ascontiguousarray(xT[b])
        m["xq"] = np.ascontiguousarray(xT[b][:, off:off + TOK])
        in_maps.append(m)
    return in_maps


def _assemble(results):
    out = np.empty((B, N, C), np.float32)
    for c in range(8):
        b, off = c // 2, (c % 2) * TOK
        out[b, off:off + TOK, :] = results[c]["outT"].T
    return out


def kernel(**inputs) -> np.ndarray:
    nc = _CACHE.get("nc")
    if nc is None:
        nc = build()
        _CACHE["nc"] = nc
    in_maps = _prep_inputs(inputs)
    res = bass_utils.run_bass_kernel_spmd(nc, in_maps, core_ids=list(range(8)))
    return _assemble(res.results)
